# revision 1
# baseline (speedup 1.0000x reference)
"""Trainium2 Bass kernel for a 2-layer GCN fingerprint network.

    h   = relu(x @ W_i + b_i)                  [N, 128] -> [N, 64]
    z   = gcn_conv(h, edge_index, W_c)         scatter/gather over E edges
    h2  = relu(z @ W_h + b_h)
    out = h2 @ W_o + b_o                       [N, 1]

Strategy (8 NeuronCores, full input in / full output out):
  - The per-edge norm factors into per-node scales: with dis = outdeg^-0.5,
      y   = dis * ((relu(x@W_i+b_i)) @ W_c)          (per-node)
      z_d = dis_d * sum_{e: col(e)=d} y[row(e)]      (gather + segment sum)
  - Phase A (replicated on every core): compute the full y table [NPAD, 64]
    fp32 into DRAM.  x is host-pretransposed to bf16 [128, NPAD].
  - Phase B (dst-sharded): edges sorted by destination; destinations sorted
    by (lo-degree, hi-degree) and grouped into 128-dst blocks.  Per block,
    dma_gather (int16 indices, so the y-table is addressed as a <32768-row
    "lo" part and the rest as "hi") lands edge e's 256B y-row at
    [partition = dst_rel, slot]; a halving-tree add on the vector engine
    reduces the slots; a small matmul tail (transpose, W_h, relu, W_o)
    finishes each block.
  - Pad nodes sit at BOTH ends of the column space so both the lo and hi
    table ranges contain all-zero rows for slot padding.
  - Block slot-count schedules are compile-time constants shared by all
    cores (SPMD): global blocks are dealt round-robin; K_j = max over the 8
    blocks dealt at step j (tight because blocks are degree-sorted).

The graph structure (edge_index) is known when kernel() is called, so all
index/slot layout is precomputed on the host; the device only moves floats.
"""

import sys

sys.path.insert(0, "/opt/trn_rl_repo")

from contextlib import ExitStack

import ml_dtypes
import numpy as np

import concourse.bass as bass
import concourse.tile as tile
from concourse import bacc, mybir
from concourse.bass_utils import run_bass_kernel_spmd
from concourse.masks import make_identity

F32 = mybir.dt.float32
BF16 = mybir.dt.bfloat16
I16 = mybir.dt.int16
AF = mybir.ActivationFunctionType

N_CORES = 8
P = 128
NODE_TILE = 512   # nodes per phase-A tile (4 groups of 128)
SPLIT = 32768     # int16 index limit for dma_gather


def _table_row(c):
    """Column id -> row in the DRAM y-table (phase-A write order)."""
    c = np.asarray(c)
    t = c // NODE_TILE
    rem = c % NODE_TILE
    g = rem // P
    p = rem % P
    return (t * NODE_TILE + p * (NODE_TILE // P) + g).astype(np.int64)


def _pack_idxs(arr):
    """[128, K] slot-layout values -> dma_gather idx tile [128, 8*K] int16.

    Position i = slot*128 + p must live at idx[i%16, i//16], replicated
    across the 8 vertical 16-partition groups.
    """
    p128, K = arr.shape
    assert p128 == P
    if K == 0:
        return np.zeros((P, 0), np.int16)
    w = arr.reshape(8, 16, K).transpose(1, 2, 0).reshape(16, 8 * K)
    return np.tile(w, (8, 1)).astype(np.int16)


def _host_prep(x, edge_index, W_i, b_i, W_c, W_h, b_h, W_o, b_o):
    """Returns (in_maps, meta) for run_bass_kernel_spmd."""
    n = x.shape[0]
    npad = -(-(n + 160) // 1024) * 1024
    pad_lo = (npad - n) // 2
    nblkg = npad // P          # global 128-dst blocks
    nblk = nblkg // N_CORES    # blocks per core
    n_tiles = npad // NODE_TILE
    if npad > SPLIT:
        # hi pads (last pad columns) must land in the hi table range
        assert npad - (npad - n - pad_lo) >= SPLIT + 0 and npad >= SPLIT + 512

    row = np.concatenate([edge_index[0], np.arange(n)]).astype(np.int64)
    col = np.concatenate([edge_index[1], np.arange(n)]).astype(np.int64)

    outdeg = np.bincount(row, minlength=n).astype(np.float64)
    dis = (outdeg ** -0.5).astype(np.float32)
    dis_pad = np.zeros(npad, np.float32)   # dst-id space
    dis_pad[:n] = dis

    src_trow = _table_row(row + pad_lo)    # y-table row of each edge's source
    is_hi = src_trow >= SPLIT
    nlo = np.bincount(col[~is_hi], minlength=n)
    nhi = np.bincount(col[is_hi], minlength=n)
    nlo_pad = np.zeros(npad, np.int64)
    nlo_pad[:n] = nlo
    nhi_pad = np.zeros(npad, np.int64)
    nhi_pad[:n] = nhi

    # destination order: lex by (lo-degree desc, hi-degree desc)
    order = np.lexsort((-nhi_pad, -nlo_pad))
    pos = np.empty(npad, np.int64)
    pos[order] = np.arange(npad)
    dst_gp = order.reshape(nblkg, P)       # [global block, partition] -> dst

    # per-edge slot within (dst, lo/hi): sort by col with lo edges first
    e_order = np.lexsort((is_hi.astype(np.int8), col))
    colg = col[e_order]
    trowg = src_trow[e_order]
    ishig = is_hi[e_order]
    starts = np.searchsorted(colg, np.arange(n))
    within = np.arange(len(colg)) - starts[colg]
    slot = np.where(ishig, within - nlo[colg], within)

    # slot-count schedules, uniform across cores; deal global blocks to
    # (core, step) in (klo, khi) lex order so the max-over-8 stays tight
    klo_g = nlo_pad[order].reshape(nblkg, P).max(1)
    khi_g = nhi_pad[order].reshape(nblkg, P).max(1)
    blk_perm = np.lexsort((-khi_g, -klo_g))
    deal_core = np.empty(nblkg, np.int64)
    deal_j = np.empty(nblkg, np.int64)
    deal_core[blk_perm] = np.arange(nblkg) % N_CORES
    deal_j[blk_perm] = np.arange(nblkg) // N_CORES
    klo = klo_g[blk_perm].reshape(nblk, N_CORES).max(1).astype(np.int64)
    khi = khi_g[blk_perm].reshape(nblk, N_CORES).max(1).astype(np.int64)
    baseL = np.concatenate([[0], np.cumsum(klo)])
    baseH = np.concatenate([[0], np.cumsum(khi)])
    SL, SH = int(baseL[-1]), int(baseH[-1])

    # slot-layout value arrays, pads pointed at all-zero pad-node rows
    pad_hi_cnt = npad - n - pad_lo
    lo_pad_rows = _table_row(np.arange(pad_lo))
    vlo = np.empty((N_CORES, P, max(SL, 1)), np.int64)
    vlo[:, :, :] = lo_pad_rows[np.arange(P * max(SL, 1)) % pad_lo].reshape(
        P, max(SL, 1))[None]
    if SH > 0:
        hi_pad_rows = _table_row(npad - pad_hi_cnt + np.arange(pad_hi_cnt)) - SPLIT
        assert (hi_pad_rows >= 0).all()
        vhi = np.empty((N_CORES, P, SH), np.int64)
        vhi[:, :, :] = hi_pad_rows[np.arange(P * SH) % pad_hi_cnt].reshape(
            P, SH)[None]
    else:
        vhi = np.zeros((N_CORES, P, 0), np.int64)

    pe = pos[colg]
    p_e = pe % P
    g_e = pe // P
    c_e = deal_core[g_e]
    j_e = deal_j[g_e]
    lo_m = ~ishig
    vlo[c_e[lo_m], p_e[lo_m], baseL[j_e[lo_m]] + slot[lo_m]] = trowg[lo_m]
    hi_m = ishig
    vhi[c_e[hi_m], p_e[hi_m], baseH[j_e[hi_m]] + slot[hi_m]] = trowg[hi_m] - SPLIT
    assert vlo.max() < SPLIT and (SH == 0 or vhi.max() < SPLIT)

    # pack to dma_gather idx layout, concatenated per block along free dim
    def pack_core(v, k_sched, base):
        parts = [
            _pack_idxs(v[:, int(base[j]): int(base[j]) + int(k_sched[j])])
            for j in range(nblk)
        ]
        return np.concatenate(parts, axis=1) if parts else np.zeros((P, 0), np.int16)

    idxs_lo = np.stack([pack_core(vlo[c][:, :SL], klo, baseL) for c in range(N_CORES)])
    idxs_hi = np.stack([pack_core(vhi[c], khi, baseH) for c in range(N_CORES)])

    def _pad_w(a, w):
        # empty/narrow inputs become HLO constants, which bass_jit rejects
        if a.shape[2] >= w:
            return a
        out = np.zeros((a.shape[0], a.shape[1], w), np.int16)
        out[:, :, : a.shape[2]] = a
        return out

    idxs_lo = _pad_w(idxs_lo, 16)
    idxs_hi = _pad_w(idxs_hi, 16)

    # phase-A per-column scale, laid out [p, t*4+g] to match mm2 groups
    disx = np.zeros(npad, np.float32)      # column space
    disx[pad_lo: pad_lo + n] = dis
    cc = np.arange(npad)
    disA = np.zeros((P, npad // P), np.float32)
    disA[cc % P, (cc // NODE_TILE) * (NODE_TILE // P) + (cc % NODE_TILE) // P] = (
        disx[cc]
    )

    # phase-B per-dst scale, per core [p, j]
    disB_all = dis_pad[dst_gp]             # [nblkg, P]
    disB = np.stack([
        disB_all[blk_perm[np.arange(nblk) * N_CORES + c]].T
        for c in range(N_CORES)
    ])
    disB = np.ascontiguousarray(disB, dtype=np.float32)

    # host-pretransposed, padded, bf16 x (real nodes at columns pad_lo..)
    xT = np.zeros((P, npad), ml_dtypes.bfloat16)
    xT[:, pad_lo: pad_lo + n] = np.ascontiguousarray(x.T).astype(ml_dtypes.bfloat16)

    shared = {
        "xT": xT,
        "W_i": W_i.astype(ml_dtypes.bfloat16),
        "W_c": W_c.astype(ml_dtypes.bfloat16),
        "W_h": W_h.astype(np.float32),
        "W_o": W_o.astype(np.float32),
        "b_i": b_i.astype(np.float32).reshape(-1, 1),
        "b_h": b_h.astype(np.float32).reshape(-1, 1),
        "disA": disA,
    }
    in_maps = [
        {**shared, "idxs_lo": idxs_lo[c], "idxs_hi": idxs_hi[c], "disB": disB[c]}
        for c in range(N_CORES)
    ]

    meta = {
        "n": n,
        "npad": npad,
        "nblk": nblk,
        "n_tiles": n_tiles,
        "klo": klo,
        "khi": khi,
        "baseL": baseL,
        "baseH": baseH,
        "SL": SL,
        "SH": SH,
        "dst_gp": dst_gp,
        "blk_perm": blk_perm,
        "b_o": float(np.asarray(b_o).reshape(-1)[0]),
        "in_dim": x.shape[1],
        "hid": W_i.shape[1],
        "debug_ytab": False,
    }
    return in_maps, meta


def _build(meta):
    npad = meta["npad"]
    nblk = meta["nblk"]
    n_tiles = meta["n_tiles"]
    klo, khi = meta["klo"], meta["khi"]
    baseL, baseH = meta["baseL"], meta["baseH"]
    SL, SH = meta["SL"], meta["SH"]
    in_dim = meta["in_dim"]
    hid = meta["hid"]
    grp = NODE_TILE // P  # mm2 groups per phase-A tile

    nc = bacc.Bacc()
    xT = nc.declare_dram_parameter("xT", [in_dim, npad], BF16, isOutput=False)
    W_i = nc.declare_dram_parameter("W_i", [in_dim, hid], BF16, isOutput=False)
    W_c = nc.declare_dram_parameter("W_c", [hid, hid], BF16, isOutput=False)
    W_h = nc.declare_dram_parameter("W_h", [hid, hid], F32, isOutput=False)
    W_o = nc.declare_dram_parameter("W_o", [hid, 1], F32, isOutput=False)
    b_i = nc.declare_dram_parameter("b_i", [hid, 1], F32, isOutput=False)
    b_h = nc.declare_dram_parameter("b_h", [hid, 1], F32, isOutput=False)
    disA = nc.declare_dram_parameter("disA", [P, npad // P], F32, isOutput=False)
    disB = nc.declare_dram_parameter("disB", [P, nblk], F32, isOutput=False)
    dil = nc.declare_dram_parameter("idxs_lo", [P, max(8 * SL, 16)], I16,
                                    isOutput=False)
    dih = nc.declare_dram_parameter("idxs_hi", [P, max(8 * SH, 16)], I16,
                                    isOutput=False)
    out = nc.declare_dram_parameter("out", [1, nblk * P], F32, isOutput=True)

    if meta["debug_ytab"]:
        ytab = nc.declare_dram_parameter("ytab", [npad, hid], F32, isOutput=True)
    else:
        ytab = nc.dram_tensor("ytab", [npad, hid], F32)

    with tile.TileContext(nc) as tc, ExitStack() as ctx:
        singles = ctx.enter_context(tc.tile_pool(name="singles", bufs=1))
        sWi = singles.tile([in_dim, hid], BF16)
        sWc = singles.tile([hid, hid], BF16)
        sWh = singles.tile([hid, hid], F32)
        sWo = singles.tile([hid, 1], F32)
        sbi = singles.tile([hid, 1], F32)
        sbh = singles.tile([hid, 1], F32)
        sdisA = singles.tile([P, npad // P], F32)
        sdisB = singles.tile([P, nblk], F32)
        sil = singles.tile([P, max(8 * SL, 16)], I16)
        sih = singles.tile([P, max(8 * SH, 16)], I16)
        ident = singles.tile([P, P], F32)
        outrow = singles.tile([1, nblk * P], F32)
        loads = [
            (sWi, W_i), (sWc, W_c), (sWh, W_h), (sWo, W_o),
            (sbi, b_i), (sbh, b_h), (sdisA, disA), (sdisB, disB),
        ]
        if SL > 0:
            loads.append((sil, dil))
        if SH > 0:
            loads.append((sih, dih))
        for dst_t, src_t in loads:
            nc.sync.dma_start(out=dst_t[:], in_=src_t[:])
        make_identity(nc, ident[:])

        # ---- Phase A: y table ----
        with (
            tc.tile_pool(name="pa_x", bufs=3) as pax,
            tc.tile_pool(name="pa_ps1", bufs=2, space="PSUM") as ps1,
            tc.tile_pool(name="pa_h", bufs=3) as pah,
            tc.tile_pool(name="pa_ps2", bufs=4, space="PSUM") as ps2,
            tc.tile_pool(name="pa_y", bufs=3) as pay,
        ):
            for t in range(n_tiles if not meta.get("skip_phaseA", False) else 0):
                xt = pax.tile([in_dim, NODE_TILE], BF16)
                nc.sync.dma_start(
                    out=xt[:], in_=xT[:, t * NODE_TILE:(t + 1) * NODE_TILE]
                )
                hps = ps1.tile([hid, NODE_TILE], F32)
                nc.tensor.matmul(hps[:], lhsT=sWi[:], rhs=xt[:], start=True, stop=True)
                ht = pah.tile([hid, NODE_TILE], BF16)
                nc.scalar.activation(ht[:], hps[:], AF.Relu, bias=sbi[:])
                yst = pay.tile([P, grp, hid], F32)
                for g in range(grp):
                    yps = ps2.tile([P, hid], F32)
                    nc.tensor.matmul(
                        yps[:],
                        lhsT=ht[:, g * P:(g + 1) * P],
                        rhs=sWc[:],
                        start=True,
                        stop=True,
                    )
                    nc.vector.tensor_scalar_mul(
                        yst[:, g, :], yps[:],
                        sdisA[:, t * grp + g: t * grp + g + 1],
                    )
                nc.sync.dma_start(
                    out=ytab[t * NODE_TILE:(t + 1) * NODE_TILE, :].rearrange(
                        "(p g) d -> p g d", p=P
                    ),
                    in_=yst[:],
                )

        # ---- Phase B: gather + segment reduce + output head ----
        skip_b = meta.get("skip_phaseB", False)
        def reduce_slots(G, k):
            while k > 1:
                k2 = k // 2
                h = k - k2
                nc.vector.tensor_add(G[:, :k2, :], G[:, :k2, :], G[:, h:h + k2, :])
                k = h

        zacc = singles.tile([P, nblk * hid], F32)
        with (
            tc.tile_pool(name="pb_gl", bufs=3) as pbgl,
            tc.tile_pool(name="pb_gh", bufs=3) as pbgh,
            tc.tile_pool(name="pb_z", bufs=3) as pbz,
            tc.tile_pool(name="pb_pst", bufs=2, space="PSUM") as pbt,
            tc.tile_pool(name="pb_r", bufs=4) as pbr,
            tc.tile_pool(name="pb_ps2", bufs=2, space="PSUM") as pb2,
            tc.tile_pool(name="pb_pso", bufs=2, space="PSUM") as pbo,
        ):
            # pass 1: lo gathers only touch table rows < SPLIT, so they can
            # start while phase A is still writing the hi half of the table
            for j in range(nblk if not skip_b else 0):
                KL = int(klo[j])
                zsl = zacc[:, j * hid:(j + 1) * hid]
                if KL > 0:
                    GL = pbgl.tile([P, KL, hid], F32, tag="gatherlo")
                    nc.gpsimd.dma_gather(
                        out_ap=GL[:],
                        in_ap=ytab[0:min(SPLIT, npad), :],
                        idxs_ap=sil[:, 8 * int(baseL[j]): 8 * (int(baseL[j]) + KL)],
                        num_idxs=P * KL,
                        num_idxs_reg=P * KL,
                        elem_size=hid,
                        single_packet=(P * KL <= 1024),
                    )
                    if not meta.get("phaseB_noReduce", False):
                        reduce_slots(GL, KL)
                    nc.vector.tensor_copy(zsl, GL[:, 0, :])
                else:
                    nc.vector.memset(zsl, 0.0)
            # pass 2: hi gathers, combine with lo partials, output head
            for j in range(nblk if not skip_b else 0):
                KH = int(khi[j])
                zsl = zacc[:, j * hid:(j + 1) * hid]
                GH = None
                if KH > 0:
                    GH = pbgh.tile([P, KH, hid], F32, tag="gatherhi")
                    nc.gpsimd.dma_gather(
                        out_ap=GH[:],
                        in_ap=ytab[SPLIT:npad, :],
                        idxs_ap=sih[:, 8 * int(baseH[j]): 8 * (int(baseH[j]) + KH)],
                        num_idxs=P * KH,
                        num_idxs_reg=P * KH,
                        elem_size=hid,
                        single_packet=(P * KH <= 1024),
                    )
                    if not meta.get("phaseB_noReduce", False):
                        reduce_slots(GH, KH)
                zs = pbz.tile([P, hid], F32)
                if GH is not None:
                    zsum = pbz.tile([P, hid], F32, tag="zsum")
                    nc.vector.tensor_add(zsum[:], zsl, GH[:, 0, :])
                else:
                    zsum = zsl
                nc.scalar.activation(
                    zs[:], zsum[:], AF.Copy, scale=sdisB[:, j:j + 1]
                )
                if meta.get("phaseB_noTail", False):
                    nc.vector.tensor_copy(
                        outrow[:, j * P: j * P + hid], zs[0:1, :]
                    )
                    continue
                pt = pbt.tile([hid, P], F32)
                nc.tensor.transpose(pt[:], zs[:], ident[:])
                rt = pbr.tile([hid, P], F32)
                nc.vector.tensor_copy(rt[:], pt[:])
                h2ps = pb2.tile([hid, P], F32)
                nc.tensor.matmul(h2ps[:], lhsT=sWh[:], rhs=rt[:], start=True, stop=True)
                h2 = pbr.tile([hid, P], F32, tag="h2")
                nc.scalar.activation(h2[:], h2ps[:], AF.Relu, bias=sbh[:])
                ops = pbo.tile([1, P], F32)
                nc.tensor.matmul(ops[:], lhsT=sWo[:], rhs=h2[:], start=True, stop=True)
                nc.scalar.activation(
                    outrow[:, j * P:(j + 1) * P], ops[:], AF.Identity,
                    bias=float(meta["b_o"]),
                )
        if skip_b:
            nc.vector.memset(outrow[:], 0.0)
        nc.sync.dma_start(out=out[:], in_=outrow[:])

    nc.finalize()
    return nc


def _assemble(results, meta):
    n = meta["n"]
    npad = meta["npad"]
    nblk = meta["nblk"]
    dst_gp = meta["dst_gp"]
    blk_perm = meta["blk_perm"]
    out_full = np.zeros(npad, np.float32)
    for c in range(N_CORES):
        vals = np.asarray(results[c]["out"]).reshape(nblk * P)
        gb = blk_perm[np.arange(nblk) * N_CORES + c]
        out_full[dst_gp[gb].ravel()] = vals
    return out_full[:n].reshape(n, 1).astype(np.float32)


def kernel(x, edge_index, W_i, b_i, W_c, W_h, b_h, W_o, b_o):
    x = np.asarray(x)
    edge_index = np.asarray(edge_index)
    in_maps, meta = _host_prep(
        x, edge_index,
        np.asarray(W_i), np.asarray(b_i), np.asarray(W_c),
        np.asarray(W_h), np.asarray(b_h), np.asarray(W_o), np.asarray(b_o),
    )
    nc = _build(meta)
    res = run_bass_kernel_spmd(nc, in_maps, list(range(N_CORES)))
    return _assemble(res.results, meta)



# revision 6
# speedup vs baseline: 8.4980x; 8.4980x over previous
"""Trainium2 Bass kernel for a 2-layer GCN fingerprint network.

    h   = relu(x @ W_i + b_i)                  [N, 128] -> [N, 64]
    z   = gcn_conv(h, edge_index, W_c)         scatter/gather over E edges
    h2  = relu(z @ W_h + b_h)
    out = h2 @ W_o + b_o                       [N, 1]

Strategy v2.1 (8 NeuronCores, full input in / full output out):

The graph is known at kernel() time, so ALL data-dependent routing is done
on the host: the host pre-orders x columns into "slot-sequence" order and
the device recomputes h per EDGE (no gather descriptors at all; the v1
dma_gather design was bottlenecked by Pool-engine descriptor generation).

  - per-edge norm factors into per-node scales: with dis = deg^-0.5,
      z_d = dis_d * sum_{e: col(e)=d} dis_src * relu(x[src] @ W_i + b_i) @ W_c
  - dis_src > 0 folds through the relu (relu(c*u) = c*relu(u)): the host
    bakes dis_src into x.  Nonzero b_i is handled by a rank-1
    (contraction-1) matmul accumulating b_i (x) disRow into the PSUM.
  - no nonlinearity sits between W_c and W_h, so W_ch = W_c @ W_h is
    precomputed on the host; the per-dst dis_d scale commutes to the very
    end (relu(c*v + b) = c*relu(v + b/c)).
  - destinations are sorted by in-degree and grouped into 128-dst blocks;
    block j gets K_j slots (max in-degree over the 8 blocks dealt at step
    j; schedule shared by all cores so the SPMD program is identical).
  - A/B partition packing: slots are split into an A half and a B half.
    One PSUM tile [128, 512] holds h for 512 A-entries on partitions 0:64
    and 512 B-entries on partitions 64:128, via two accumulating matmuls
    with zero-padded stationaries [W_i | 0] and [0 | W_i].  All downstream
    vector/scalar ops then run at full 128-partition width.
  - relu + segment-sum are fused: scalar_tensor_tensor computes
    AG += max(psum, 0) per chunk (bf16 accumulator), then two halving adds
    collapse AG's 4 slot-columns; the A-half/B-half merge is folded into
    the tail matmul with a stacked stationary [W_ch ; W_ch].
  - the tail (W_ch, relu, W_o, * dis_d) runs once, batched over all 49
    blocks at 512-wide, entirely in bf16 (fp32 matmuls are 4x slower).

Per-core traffic is the ~28MB xseq stream; everything else is on-chip.
"""

import sys

sys.path.insert(0, "/opt/trn_rl_repo")

from contextlib import ExitStack

import ml_dtypes
import numpy as np

import concourse.bass as bass
import concourse.tile as tile
from concourse import bacc, mybir
from concourse.bass_utils import run_bass_kernel_spmd

F32 = mybir.dt.float32
BF16 = mybir.dt.bfloat16
AF = mybir.ActivationFunctionType
ALU = mybir.AluOpType

N_CORES = 8
P = 128
MMF = 512          # matmul moving free dim (4 slots of 128)


def _host_prep(x, edge_index, W_i, b_i, W_c, W_h, b_h, W_o, b_o):
    """Returns (in_maps, meta) for run_bass_kernel_spmd."""
    n, in_dim = x.shape
    hid = W_i.shape[1]
    npad = -(-n // 1024) * 1024
    nblkg = npad // P
    assert nblkg % N_CORES == 0
    nblk = nblkg // N_CORES

    row = np.concatenate([edge_index[0], np.arange(n)]).astype(np.int64)
    col = np.concatenate([edge_index[1], np.arange(n)]).astype(np.int64)

    outdeg = np.bincount(row, minlength=n).astype(np.float64)
    dis = (outdeg ** -0.5).astype(np.float32)   # deg >= 1 (self loops)

    indeg = np.bincount(col, minlength=npad)
    order = np.argsort(-indeg, kind="stable")   # dsts by in-degree desc
    dst_gp = order.reshape(nblkg, P)            # [global block, partition]
    kblk = indeg[order].reshape(nblkg, P).max(1)
    # blocks are in degree order; deal round-robin: step j gets blocks
    # j*8 .. j*8+7, K_j = max over them (tight since sorted)
    K = kblk.reshape(nblk, N_CORES).max(1).astype(np.int64)
    K = np.maximum(K, 1)
    KH = -(-K // 2)                              # A/B pair-slots per block
    CW = 2 * KH * P                              # xseq columns per block
    cbase = np.concatenate([[0], np.cumsum(CW)])
    Ltot = int(cbase[-1])

    # edges sorted by destination; starts[d] = first edge of dst d
    e_order = np.argsort(col, kind="stable")
    csrc = row[e_order]
    starts = np.searchsorted(col[e_order], np.arange(npad))

    # per-(step, slot) tables, slot s of block j lives at xseq column
    #   cbase[j] + (sp // 4) * 1024 + half * wt + (sp % 4) * 128 + p
    # where sp = s if s < KH[j] (A half) else s - KH[j] (B half) and wt is
    # the chunk width (512, except the last partial chunk of a block)
    SKtot = int(K.sum())
    row_j = np.repeat(np.arange(nblk), K)            # [SKtot]
    row_s = np.arange(SKtot) - np.repeat(np.cumsum(K) - K, K)
    khj = KH[row_j]
    half = (row_s >= khj).astype(np.int64)
    sp = row_s - half * khj                          # pair-slot index
    wt = np.minimum(MMF, (khj - (sp // 4) * 4) * P)  # chunk width
    colpos = cbase[row_j] + (sp // 4) * 1024 + half * wt + (sp % 4) * P

    # dis-prescaled, transposed x with a zero pad column at index n
    xs_T = np.zeros((in_dim, n + 1), ml_dtypes.bfloat16)
    xs_T[:, :n] = (x.T * dis[None, :]).astype(ml_dtypes.bfloat16)

    dis_pad = np.zeros(npad, np.float32)
    dis_pad[:n] = dis

    has_bi = bool(np.any(np.asarray(b_i)))
    has_bh = bool(np.any(np.asarray(b_h)))

    in_maps = []
    gbs = []
    for c in range(N_CORES):
        gb = np.arange(nblk) * N_CORES + c           # global block ids
        gbs.append(gb)
        dsts = dst_gp[gb]                            # [nblk, P]
        dst_mat = dsts[row_j]                        # [SKtot, P]
        deg_mat = indeg[dst_mat]
        mask = row_s[:, None] < deg_mat              # valid slot?
        eidx = starts[dst_mat] + row_s[:, None]
        seq = np.where(mask, csrc[np.minimum(eidx, len(csrc) - 1)], n)
        seq_cols = np.full(Ltot, n, np.int64)        # default: zero pad col
        seq_cols[(colpos[:, None] + np.arange(P)).reshape(-1)] = seq.reshape(-1)
        xseq = np.ascontiguousarray(xs_T[:, seq_cols])
        dRow = dis_pad[dsts].reshape(1, nblk * P).astype(np.float32)
        m = {"xseq": xseq, "dRow": np.ascontiguousarray(dRow)}
        if has_bi:
            dseq_e = np.where(mask, dis[np.minimum(seq, n - 1)], 0.0)
            dseq = np.zeros(Ltot, np.float32)
            dseq[(colpos[:, None] + np.arange(P)).reshape(-1)] = dseq_e.reshape(-1)
            m["disSeq"] = dseq.reshape(1, Ltot)
        if has_bh:
            with np.errstate(divide="ignore"):
                invd = np.where(dRow > 0, 1.0 / np.maximum(dRow, 1e-30), 0.0)
            m["invdRow"] = invd.astype(np.float32)
        in_maps.append(m)

    W_ch = (np.asarray(W_c, np.float64) @ np.asarray(W_h, np.float64))
    Wi64 = np.asarray(W_i, np.float64)
    W_iA = np.concatenate([Wi64, np.zeros_like(Wi64)], axis=1)  # [W_i | 0]
    W_iB = np.concatenate([np.zeros_like(Wi64), Wi64], axis=1)  # [0 | W_i]
    W_chAB = np.concatenate([W_ch, W_ch], axis=0)               # [W_ch ; W_ch]
    shared = {
        "W_iA": np.ascontiguousarray(W_iA).astype(ml_dtypes.bfloat16),
        "W_iB": np.ascontiguousarray(W_iB).astype(ml_dtypes.bfloat16),
        "W_chAB": np.ascontiguousarray(W_chAB).astype(ml_dtypes.bfloat16),
        "W_o": np.asarray(W_o).astype(ml_dtypes.bfloat16),
    }
    if has_bi:
        shared["b_i"] = np.asarray(b_i, np.float32).reshape(1, hid)
    if has_bh:
        shared["b_h"] = np.asarray(b_h, np.float32).reshape(1, hid)
    for m in in_maps:
        m.update(shared)

    meta = {
        "n": n,
        "npad": npad,
        "nblk": nblk,
        "K": K,
        "KH": KH,
        "cbase": cbase,
        "Ltot": Ltot,
        "in_dim": in_dim,
        "hid": hid,
        "dst_gp": dst_gp,
        "gbs": gbs,
        "has_bi": has_bi,
        "has_bh": has_bh,
        "b_o": float(np.asarray(b_o).reshape(-1)[0]),
    }
    return in_maps, meta


def _build(meta):
    nblk = meta["nblk"]
    KH = meta["KH"]
    cbase = meta["cbase"]
    Ltot = meta["Ltot"]
    in_dim = meta["in_dim"]
    hid = meta["hid"]
    has_bi = meta["has_bi"]
    has_bh = meta["has_bh"]
    b_o = meta["b_o"]
    khmax = int(KH.max())
    NO = nblk * P                                  # output columns

    nc = bacc.Bacc()
    xseq = nc.declare_dram_parameter("xseq", [in_dim, Ltot], BF16, isOutput=False)
    W_iA = nc.declare_dram_parameter("W_iA", [in_dim, 2 * hid], BF16, isOutput=False)
    W_iB = nc.declare_dram_parameter("W_iB", [in_dim, 2 * hid], BF16, isOutput=False)
    W_chAB = nc.declare_dram_parameter("W_chAB", [2 * hid, hid], BF16,
                                       isOutput=False)
    W_o = nc.declare_dram_parameter("W_o", [hid, 1], BF16, isOutput=False)
    dRow = nc.declare_dram_parameter("dRow", [1, NO], F32, isOutput=False)
    if has_bi:
        b_i = nc.declare_dram_parameter("b_i", [1, hid], F32, isOutput=False)
        disSeq = nc.declare_dram_parameter("disSeq", [1, Ltot], F32, isOutput=False)
    if has_bh:
        b_h = nc.declare_dram_parameter("b_h", [1, hid], F32, isOutput=False)
        invdRow = nc.declare_dram_parameter("invdRow", [1, NO], F32, isOutput=False)
    out = nc.declare_dram_parameter("out", [1, NO], F32, isOutput=True)

    with tile.TileContext(nc) as tc, ExitStack() as ctx:
        singles = ctx.enter_context(tc.tile_pool(name="singles", bufs=1))
        sWiA = singles.tile([in_dim, 2 * hid], BF16)
        sWiB = singles.tile([in_dim, 2 * hid], BF16)
        sWch = singles.tile([2 * hid, hid], BF16)
        sWo = singles.tile([hid, 1], BF16)
        sdR = singles.tile([1, NO], F32)
        zall = singles.tile([2 * hid, NO], BF16)
        outrow = singles.tile([1, NO], F32)
        loads = [(sWiA, W_iA), (sWiB, W_iB), (sWch, W_chAB), (sWo, W_o),
                 (sdR, dRow)]
        if has_bi:
            sbi = singles.tile([1, hid], F32)
            sdis = singles.tile([1, Ltot], F32)
            loads += [(sbi, b_i), (sdis, disSeq)]
        if has_bh:
            sbh = singles.tile([1, hid], F32)
            sinvd = singles.tile([1, NO], F32)
            loads += [(sbh, b_h), (sinvd, invdRow)]
        for dst_t, src_t in loads:
            nc.sync.dma_start(out=dst_t[:], in_=src_t[:])

        with (
            tc.tile_pool(name="px", bufs=3) as px,
            tc.tile_pool(name="pps", bufs=4, space="PSUM") as pps,
            tc.tile_pool(name="pag", bufs=3) as pag,
            tc.tile_pool(name="ph", bufs=2) as ph,
            tc.tile_pool(name="ps2", bufs=2, space="PSUM") as ps2,
            tc.tile_pool(name="pso", bufs=2, space="PSUM") as pso,
        ):
            for j in range(nblk):
                KHj = int(KH[j])
                off = int(cbase[j])
                L = 2 * KHj * P                    # block columns (A+B)
                xb = px.tile([in_dim, 2 * khmax * P], BF16, tag="xb")
                nc.sync.dma_start(out=xb[:, :L], in_=xseq[:, off: off + L])
                AG = pag.tile([P, MMF], BF16, tag="ag")
                nchunk = -(-KHj // 4)
                for t in range(nchunk):
                    w = min(MMF, KHj * P - t * MMF)
                    ca = t * 1024                  # A cols of this chunk
                    ps = pps.tile([P, MMF], F32)
                    nc.tensor.matmul(
                        ps[:, :w], lhsT=sWiA[:], rhs=xb[:, ca: ca + w],
                        start=True, stop=False,
                    )
                    nc.tensor.matmul(
                        ps[:, :w], lhsT=sWiB[:],
                        rhs=xb[:, ca + w: ca + 2 * w],
                        start=False, stop=not has_bi,
                    )
                    if has_bi:
                        # rank-1 bias: A then B half (disSeq is column-matched)
                        nc.tensor.matmul(
                            ps[:, :w], lhsT=sbi[:],
                            rhs=sdis[:, off + ca: off + ca + w],
                            start=False, stop=False,
                        )
                        nc.tensor.matmul(
                            ps[:, :w], lhsT=sbi[:],
                            rhs=sdis[:, off + ca + w: off + ca + 2 * w],
                            start=False, stop=True,
                        )
                    if t == 0:
                        nc.scalar.activation(AG[:, :w], ps[:, :w],
                                             AF.Relu, bias=0.0)
                    else:
                        nc.vector.scalar_tensor_tensor(
                            AG[:, :w], ps[:, :w], 0.0, AG[:, :w],
                            op0=ALU.max, op1=ALU.add,
                        )
                # collapse AG's remaining (up to 4) slot-columns
                k = min(KHj, 4)
                while k > 1:
                    k2 = k // 2
                    h = k - k2
                    nc.vector.tensor_add(
                        AG[:, : k2 * P], AG[:, : k2 * P],
                        AG[:, h * P: (h + k2) * P],
                    )
                    k = h
                nc.vector.tensor_copy(zall[:, j * P: (j + 1) * P], AG[:, :P])

            # batched tail over all blocks: W_ch (+A/B merge), relu, W_o, *dis
            for t in range(0, NO, MMF):
                w = min(MMF, NO - t)
                p2 = ps2.tile([hid, MMF], F32)
                nc.tensor.matmul(p2[:, :w], lhsT=sWch[:], rhs=zall[:, t: t + w],
                                 start=True, stop=not has_bh)
                if has_bh:
                    nc.tensor.matmul(p2[:, :w], lhsT=sbh[:],
                                     rhs=sinvd[:, t: t + w],
                                     start=False, stop=True)
                h2 = ph.tile([hid, MMF], BF16)
                nc.scalar.activation(h2[:, :w], p2[:, :w], AF.Relu, bias=0.0)
                po = pso.tile([1, MMF], F32)
                nc.tensor.matmul(po[:, :w], lhsT=sWo[:], rhs=h2[:, :w],
                                 start=True, stop=True)
                nc.vector.tensor_mul(outrow[:, t: t + w], po[:, :w],
                                     sdR[:, t: t + w])
                if b_o != 0.0:
                    nc.vector.tensor_scalar_add(
                        outrow[:, t: t + w], outrow[:, t: t + w], b_o,
                    )
        nc.sync.dma_start(out=out[:], in_=outrow[:])

    nc.finalize()
    return nc


def _assemble(results, meta):
    n = meta["n"]
    npad = meta["npad"]
    nblk = meta["nblk"]
    dst_gp = meta["dst_gp"]
    out_full = np.zeros(npad, np.float32)
    for c in range(N_CORES):
        vals = np.asarray(results[c]["out"]).reshape(nblk * P)
        out_full[dst_gp[meta["gbs"][c]].ravel()] = vals
    return out_full[:n].reshape(n, 1).astype(np.float32)


def kernel(x, edge_index, W_i, b_i, W_c, W_h, b_h, W_o, b_o):
    x = np.asarray(x)
    edge_index = np.asarray(edge_index)
    in_maps, meta = _host_prep(
        x, edge_index,
        np.asarray(W_i), np.asarray(b_i), np.asarray(W_c),
        np.asarray(W_h), np.asarray(b_h), np.asarray(W_o), np.asarray(b_o),
    )
    nc = _build(meta)
    res = run_bass_kernel_spmd(nc, in_maps, list(range(N_CORES)))
    return _assemble(res.results, meta)


# revision 9
# speedup vs baseline: 9.4688x; 1.1142x over previous
"""Trainium2 Bass kernel for a 2-layer GCN fingerprint network.

    h   = relu(x @ W_i + b_i)                  [N, 128] -> [N, 64]
    z   = gcn_conv(h, edge_index, W_c)         scatter/gather over E edges
    h2  = relu(z @ W_h + b_h)
    out = h2 @ W_o + b_o                       [N, 1]

Strategy v2.1 (8 NeuronCores, full input in / full output out):

The graph is known at kernel() time, so ALL data-dependent routing is done
on the host: the host pre-orders x columns into "slot-sequence" order and
the device recomputes h per EDGE (no gather descriptors at all; the v1
dma_gather design was bottlenecked by Pool-engine descriptor generation).

  - per-edge norm factors into per-node scales: with dis = deg^-0.5,
      z_d = dis_d * sum_{e: col(e)=d} dis_src * relu(x[src] @ W_i + b_i) @ W_c
  - dis_src > 0 folds through the relu (relu(c*u) = c*relu(u)): the host
    bakes dis_src into x.  Nonzero b_i is handled by a rank-1
    (contraction-1) matmul accumulating b_i (x) disRow into the PSUM.
  - no nonlinearity sits between W_c and W_h, so W_ch = W_c @ W_h is
    precomputed on the host; the per-dst dis_d scale commutes to the very
    end (relu(c*v + b) = c*relu(v + b/c)).
  - destinations are sorted by in-degree and grouped into 128-dst blocks;
    block j gets K_j slots (max in-degree over the 8 blocks dealt at step
    j; schedule shared by all cores so the SPMD program is identical).
  - A/B partition packing: slots are split into an A half and a B half.
    One PSUM tile [128, 512] holds h for 512 A-entries on partitions 0:64
    and 512 B-entries on partitions 64:128, via two accumulating matmuls
    with zero-padded stationaries [W_i | 0] and [0 | W_i].  All downstream
    vector/scalar ops then run at full 128-partition width.
  - relu + segment-sum are fused: scalar_tensor_tensor computes
    AG += max(psum, 0) per chunk (bf16 accumulator), then two halving adds
    collapse AG's 4 slot-columns; the A-half/B-half merge is folded into
    the tail matmul with a stacked stationary [W_ch ; W_ch].
  - the tail (W_ch, relu, W_o, * dis_d) runs once, batched over all 49
    blocks at 512-wide, entirely in bf16 (fp32 matmuls are 4x slower).

Per-core traffic is the ~28MB xseq stream; everything else is on-chip.
"""

import sys

sys.path.insert(0, "/opt/trn_rl_repo")

from contextlib import ExitStack

import ml_dtypes
import numpy as np

import concourse.bass as bass
import concourse.tile as tile
from concourse import bacc, mybir
from concourse.bass_utils import run_bass_kernel_spmd

F32 = mybir.dt.float32
BF16 = mybir.dt.bfloat16
AF = mybir.ActivationFunctionType
ALU = mybir.AluOpType

N_CORES = 8
P = 128
MMF = 512          # matmul moving free dim (4 slots of 128)


def _host_prep(x, edge_index, W_i, b_i, W_c, W_h, b_h, W_o, b_o):
    """Returns (in_maps, meta) for run_bass_kernel_spmd."""
    n, in_dim = x.shape
    hid = W_i.shape[1]
    npad = -(-n // 1024) * 1024
    nblkg = npad // P
    assert nblkg % N_CORES == 0
    nblk = nblkg // N_CORES

    row = np.concatenate([edge_index[0], np.arange(n)]).astype(np.int64)
    col = np.concatenate([edge_index[1], np.arange(n)]).astype(np.int64)

    outdeg = np.bincount(row, minlength=n).astype(np.float64)
    dis = (outdeg ** -0.5).astype(np.float32)   # deg >= 1 (self loops)

    indeg = np.bincount(col, minlength=npad)
    order = np.argsort(-indeg, kind="stable")   # dsts by in-degree desc
    dst_gp = order.reshape(nblkg, P)            # [global block, partition]
    kblk = indeg[order].reshape(nblkg, P).max(1)
    # blocks are in degree order; deal round-robin: step j gets blocks
    # j*8 .. j*8+7, K_j = max over them (tight since sorted)
    K = kblk.reshape(nblk, N_CORES).max(1).astype(np.int64)
    K = np.maximum(K, 1)
    KH = -(-K // 2)                              # A/B pair-slots per block
    CW = 2 * KH * P                              # xseq columns per block
    cbase = np.concatenate([[0], np.cumsum(CW)])
    Ltot = int(cbase[-1])

    # edges sorted by destination; starts[d] = first edge of dst d
    e_order = np.argsort(col, kind="stable")
    csrc = row[e_order]
    starts = np.searchsorted(col[e_order], np.arange(npad))

    # per-(step, slot) tables, slot s of block j lives at xseq column
    #   cbase[j] + (sp // 4) * 1024 + half * wt + (sp % 4) * 128 + p
    # where sp = s if s < KH[j] (A half) else s - KH[j] (B half) and wt is
    # the chunk width (512, except the last partial chunk of a block)
    SKtot = int(K.sum())
    row_j = np.repeat(np.arange(nblk), K)            # [SKtot]
    row_s = np.arange(SKtot) - np.repeat(np.cumsum(K) - K, K)
    khj = KH[row_j]
    half = (row_s >= khj).astype(np.int64)
    sp = row_s - half * khj                          # pair-slot index
    wt = np.minimum(MMF, (khj - (sp // 4) * 4) * P)  # chunk width
    colpos = cbase[row_j] + (sp // 4) * 1024 + half * wt + (sp % 4) * P

    # dis-prescaled, transposed x with a zero pad column at index n
    xs_T = np.zeros((in_dim, n + 1), ml_dtypes.bfloat16)
    xs_T[:, :n] = (x.T * dis[None, :]).astype(ml_dtypes.bfloat16)

    dis_pad = np.zeros(npad, np.float32)
    dis_pad[:n] = dis

    has_bi = bool(np.any(np.asarray(b_i)))
    has_bh = bool(np.any(np.asarray(b_h)))

    in_maps = []
    gbs = []
    for c in range(N_CORES):
        gb = np.arange(nblk) * N_CORES + c           # global block ids
        gbs.append(gb)
        dsts = dst_gp[gb]                            # [nblk, P]
        dst_mat = dsts[row_j]                        # [SKtot, P]
        deg_mat = indeg[dst_mat]
        mask = row_s[:, None] < deg_mat              # valid slot?
        eidx = starts[dst_mat] + row_s[:, None]
        seq = np.where(mask, csrc[np.minimum(eidx, len(csrc) - 1)], n)
        seq_cols = np.full(Ltot, n, np.int64)        # default: zero pad col
        seq_cols[(colpos[:, None] + np.arange(P)).reshape(-1)] = seq.reshape(-1)
        xseq = np.ascontiguousarray(xs_T[:, seq_cols])
        dRow = dis_pad[dsts].reshape(1, nblk * P).astype(np.float32)
        m = {"xseq": xseq, "dRow": np.ascontiguousarray(dRow)}
        if has_bi:
            dseq_e = np.where(mask, dis[np.minimum(seq, n - 1)], 0.0)
            dseq = np.zeros(Ltot, np.float32)
            dseq[(colpos[:, None] + np.arange(P)).reshape(-1)] = dseq_e.reshape(-1)
            m["disSeq"] = dseq.reshape(1, Ltot)
        if has_bh:
            with np.errstate(divide="ignore"):
                invd = np.where(dRow > 0, 1.0 / np.maximum(dRow, 1e-30), 0.0)
            m["invdRow"] = invd.astype(np.float32)
        in_maps.append(m)

    W_ch = (np.asarray(W_c, np.float64) @ np.asarray(W_h, np.float64))
    Wi64 = np.asarray(W_i, np.float64)
    W_iA = np.concatenate([Wi64, np.zeros_like(Wi64)], axis=1)  # [W_i | 0]
    W_iB = np.concatenate([np.zeros_like(Wi64), Wi64], axis=1)  # [0 | W_i]
    W_chAB = np.concatenate([W_ch, W_ch], axis=0)               # [W_ch ; W_ch]
    shared = {
        "W_iA": np.ascontiguousarray(W_iA).astype(ml_dtypes.bfloat16),
        "W_iB": np.ascontiguousarray(W_iB).astype(ml_dtypes.bfloat16),
        "W_chAB": np.ascontiguousarray(W_chAB).astype(ml_dtypes.bfloat16),
        "W_o": np.asarray(W_o).astype(ml_dtypes.bfloat16),
    }
    if has_bi:
        shared["b_i"] = np.asarray(b_i, np.float32).reshape(1, hid)
    if has_bh:
        shared["b_h"] = np.asarray(b_h, np.float32).reshape(1, hid)
    for m in in_maps:
        m.update(shared)

    meta = {
        "n": n,
        "npad": npad,
        "nblk": nblk,
        "K": K,
        "KH": KH,
        "cbase": cbase,
        "Ltot": Ltot,
        "in_dim": in_dim,
        "hid": hid,
        "dst_gp": dst_gp,
        "gbs": gbs,
        "has_bi": has_bi,
        "has_bh": has_bh,
        "b_o": float(np.asarray(b_o).reshape(-1)[0]),
    }
    return in_maps, meta


def _build(meta):
    nblk = meta["nblk"]
    KH = meta["KH"]
    cbase = meta["cbase"]
    Ltot = meta["Ltot"]
    in_dim = meta["in_dim"]
    hid = meta["hid"]
    has_bi = meta["has_bi"]
    has_bh = meta["has_bh"]
    b_o = meta["b_o"]
    khmax = int(KH.max())
    NO = nblk * P                                  # output columns

    nc = bacc.Bacc()
    xseq = nc.declare_dram_parameter("xseq", [in_dim, Ltot], BF16, isOutput=False)
    W_iA = nc.declare_dram_parameter("W_iA", [in_dim, 2 * hid], BF16, isOutput=False)
    W_iB = nc.declare_dram_parameter("W_iB", [in_dim, 2 * hid], BF16, isOutput=False)
    W_chAB = nc.declare_dram_parameter("W_chAB", [2 * hid, hid], BF16,
                                       isOutput=False)
    W_o = nc.declare_dram_parameter("W_o", [hid, 1], BF16, isOutput=False)
    dRow = nc.declare_dram_parameter("dRow", [1, NO], F32, isOutput=False)
    if has_bi:
        b_i = nc.declare_dram_parameter("b_i", [1, hid], F32, isOutput=False)
        disSeq = nc.declare_dram_parameter("disSeq", [1, Ltot], F32, isOutput=False)
    if has_bh:
        b_h = nc.declare_dram_parameter("b_h", [1, hid], F32, isOutput=False)
        invdRow = nc.declare_dram_parameter("invdRow", [1, NO], F32, isOutput=False)
    out = nc.declare_dram_parameter("out", [1, NO], F32, isOutput=True)

    with tile.TileContext(nc) as tc, ExitStack() as ctx:
        singles = ctx.enter_context(tc.tile_pool(name="singles", bufs=1))
        sWiA = singles.tile([in_dim, 2 * hid], BF16)
        sWiB = singles.tile([in_dim, 2 * hid], BF16)
        sWch = singles.tile([2 * hid, hid], BF16)
        sWo = singles.tile([hid, 1], BF16)
        sdR = singles.tile([1, NO], F32)
        zall = singles.tile([2 * hid, NO], BF16)
        outrow = singles.tile([1, NO], F32)
        loads = [(sWiA, W_iA), (sWiB, W_iB), (sWch, W_chAB), (sWo, W_o),
                 (sdR, dRow)]
        if has_bi:
            sbi = singles.tile([1, hid], F32)
            sdis = singles.tile([1, Ltot], F32)
            loads += [(sbi, b_i), (sdis, disSeq)]
        if has_bh:
            sbh = singles.tile([1, hid], F32)
            sinvd = singles.tile([1, NO], F32)
            loads += [(sbh, b_h), (sinvd, invdRow)]
        for dst_t, src_t in loads:
            nc.sync.dma_start(out=dst_t[:], in_=src_t[:])

        with (
            tc.tile_pool(name="px", bufs=3) as px,
            tc.tile_pool(name="pps", bufs=4, space="PSUM") as pps,
            tc.tile_pool(name="pag", bufs=3) as pag,
            tc.tile_pool(name="ph", bufs=2) as ph,
            tc.tile_pool(name="ps2", bufs=2, space="PSUM") as ps2,
            tc.tile_pool(name="pso", bufs=2, space="PSUM") as pso,
        ):
            def tail_chunk(t):
                # tail over 4 blocks: W_ch (+A/B merge), relu, W_o, *dis
                w = min(MMF, NO - t)
                p2 = ps2.tile([hid, MMF], F32)
                nc.tensor.matmul(p2[:, :w], lhsT=sWch[:], rhs=zall[:, t: t + w],
                                 start=True, stop=not has_bh)
                if has_bh:
                    nc.tensor.matmul(p2[:, :w], lhsT=sbh[:],
                                     rhs=sinvd[:, t: t + w],
                                     start=False, stop=True)
                h2 = ph.tile([hid, MMF], BF16)
                nc.scalar.activation(h2[:, :w], p2[:, :w], AF.Relu, bias=0.0)
                po = pso.tile([1, MMF], F32)
                nc.tensor.matmul(po[:, :w], lhsT=sWo[:], rhs=h2[:, :w],
                                 start=True, stop=True)
                nc.vector.tensor_mul(outrow[:, t: t + w], po[:, :w],
                                     sdR[:, t: t + w])
                if b_o != 0.0:
                    nc.vector.tensor_scalar_add(
                        outrow[:, t: t + w], outrow[:, t: t + w], b_o,
                    )

            for j in range(nblk):
                KHj = int(KH[j])
                off = int(cbase[j])
                L = 2 * KHj * P                    # block columns (A+B)
                xb = px.tile([in_dim, 2 * khmax * P], BF16, tag="xb")
                dmae = nc.sync if j % 2 == 0 else nc.scalar
                dmae.dma_start(out=xb[:, :L], in_=xseq[:, off: off + L])
                # Pool cannot read PSUM, so the STT chunks stay on DVE; the
                # SBUF-only collapse ops go to the otherwise-idle Pool engine
                ve = nc.gpsimd
                AG = pag.tile([P, MMF], BF16, tag="ag")
                nchunk = -(-KHj // 4)
                for t in range(nchunk):
                    w = min(MMF, KHj * P - t * MMF)
                    ca = t * 1024                  # A cols of this chunk
                    ps = pps.tile([P, MMF], F32)
                    nc.tensor.matmul(
                        ps[:, :w], lhsT=sWiA[:], rhs=xb[:, ca: ca + w],
                        start=True, stop=False,
                    )
                    nc.tensor.matmul(
                        ps[:, :w], lhsT=sWiB[:],
                        rhs=xb[:, ca + w: ca + 2 * w],
                        start=False, stop=not has_bi,
                    )
                    if has_bi:
                        # rank-1 bias: A then B half (disSeq is column-matched)
                        nc.tensor.matmul(
                            ps[:, :w], lhsT=sbi[:],
                            rhs=sdis[:, off + ca: off + ca + w],
                            start=False, stop=False,
                        )
                        nc.tensor.matmul(
                            ps[:, :w], lhsT=sbi[:],
                            rhs=sdis[:, off + ca + w: off + ca + 2 * w],
                            start=False, stop=True,
                        )
                    if t == 0:
                        nc.scalar.activation(AG[:, :w], ps[:, :w],
                                             AF.Relu, bias=0.0)
                    else:
                        nc.vector.scalar_tensor_tensor(
                            AG[:, :w], ps[:, :w], 0.0, AG[:, :w],
                            op0=ALU.max, op1=ALU.add,
                        )
                # collapse AG's remaining (up to 4) slot-columns; the last
                # halving step writes straight into zall
                zsl = zall[:, j * P: (j + 1) * P]
                k = min(KHj, 4)
                while k > 2:
                    k2 = k // 2
                    h = k - k2
                    ve.tensor_add(
                        AG[:, : k2 * P], AG[:, : k2 * P],
                        AG[:, h * P: (h + k2) * P],
                    )
                    k = h
                if k == 2:
                    ve.tensor_add(zsl, AG[:, :P], AG[:, P: 2 * P])
                else:
                    ve.tensor_copy(zsl, AG[:, :P])
                if j % 4 == 3:
                    tail_chunk((j - 3) * P)
            if nblk % 4 != 0:
                tail_chunk((nblk - nblk % 4) * P)
        nc.sync.dma_start(out=out[:], in_=outrow[:])

    nc.finalize()
    return nc


def _assemble(results, meta):
    n = meta["n"]
    npad = meta["npad"]
    nblk = meta["nblk"]
    dst_gp = meta["dst_gp"]
    out_full = np.zeros(npad, np.float32)
    for c in range(N_CORES):
        vals = np.asarray(results[c]["out"]).reshape(nblk * P)
        out_full[dst_gp[meta["gbs"][c]].ravel()] = vals
    return out_full[:n].reshape(n, 1).astype(np.float32)


def kernel(x, edge_index, W_i, b_i, W_c, W_h, b_h, W_o, b_o):
    x = np.asarray(x)
    edge_index = np.asarray(edge_index)
    in_maps, meta = _host_prep(
        x, edge_index,
        np.asarray(W_i), np.asarray(b_i), np.asarray(W_c),
        np.asarray(W_h), np.asarray(b_h), np.asarray(W_o), np.asarray(b_o),
    )
    nc = _build(meta)
    res = run_bass_kernel_spmd(nc, in_maps, list(range(N_CORES)))
    return _assemble(res.results, meta)


# revision 12
# speedup vs baseline: 10.7306x; 1.1333x over previous
"""Trainium2 Bass kernel for a 2-layer GCN fingerprint network.

    h   = relu(x @ W_i + b_i)                  [N, 128] -> [N, 64]
    z   = gcn_conv(h, edge_index, W_c)         scatter/gather over E edges
    h2  = relu(z @ W_h + b_h)
    out = h2 @ W_o + b_o                       [N, 1]

Strategy v2.1 (8 NeuronCores, full input in / full output out):

The graph is known at kernel() time, so ALL data-dependent routing is done
on the host: the host pre-orders x columns into "slot-sequence" order and
the device recomputes h per EDGE (no gather descriptors at all; the v1
dma_gather design was bottlenecked by Pool-engine descriptor generation).

  - per-edge norm factors into per-node scales: with dis = deg^-0.5,
      z_d = dis_d * sum_{e: col(e)=d} dis_src * relu(x[src] @ W_i + b_i) @ W_c
  - dis_src > 0 folds through the relu (relu(c*u) = c*relu(u)): the host
    bakes dis_src into x.  Nonzero b_i is handled by a rank-1
    (contraction-1) matmul accumulating b_i (x) disRow into the PSUM.
  - no nonlinearity sits between W_c and W_h, so W_ch = W_c @ W_h is
    precomputed on the host; the per-dst dis_d scale commutes to the very
    end (relu(c*v + b) = c*relu(v + b/c)).
  - destinations are sorted by in-degree and grouped into 128-dst blocks;
    block j gets K_j slots (max in-degree over the 8 blocks dealt at step
    j; schedule shared by all cores so the SPMD program is identical).
  - A/B partition packing: slots are split into an A half and a B half.
    One PSUM tile [128, 512] holds h for 512 A-entries on partitions 0:64
    and 512 B-entries on partitions 64:128, via two accumulating matmuls
    with zero-padded stationaries [W_i | 0] and [0 | W_i].  All downstream
    vector/scalar ops then run at full 128-partition width.
  - relu + segment-sum are fused: scalar_tensor_tensor computes
    AG += max(psum, 0) per chunk (bf16 accumulator), then two halving adds
    collapse AG's 4 slot-columns; the A-half/B-half merge is folded into
    the tail matmul with a stacked stationary [W_ch ; W_ch].
  - the tail (W_ch, relu, W_o, * dis_d) runs once, batched over all 49
    blocks at 512-wide, entirely in bf16 (fp32 matmuls are 4x slower).

Per-core traffic is the ~28MB xseq stream; everything else is on-chip.
"""

import sys

sys.path.insert(0, "/opt/trn_rl_repo")

from contextlib import ExitStack

import ml_dtypes
import numpy as np

import concourse.bass as bass
import concourse.tile as tile
from concourse import bacc, mybir
from concourse.bass_utils import run_bass_kernel_spmd

F32 = mybir.dt.float32
BF16 = mybir.dt.bfloat16
AF = mybir.ActivationFunctionType
ALU = mybir.AluOpType

N_CORES = 8
P = 128
MMF = 512          # matmul moving free dim (4 slots of 128)


def _host_prep(x, edge_index, W_i, b_i, W_c, W_h, b_h, W_o, b_o):
    """Returns (in_maps, meta) for run_bass_kernel_spmd."""
    n, in_dim = x.shape
    hid = W_i.shape[1]
    npad = -(-n // 1024) * 1024
    nblkg = npad // P
    assert nblkg % N_CORES == 0
    nblk = nblkg // N_CORES

    row = np.concatenate([edge_index[0], np.arange(n)]).astype(np.int64)
    col = np.concatenate([edge_index[1], np.arange(n)]).astype(np.int64)

    outdeg = np.bincount(row, minlength=n).astype(np.float64)
    dis = (outdeg ** -0.5).astype(np.float32)   # deg >= 1 (self loops)

    indeg = np.bincount(col, minlength=npad)
    order = np.argsort(-indeg, kind="stable")   # dsts by in-degree desc
    dst_gp = order.reshape(nblkg, P)            # [global block, partition]
    kblk = indeg[order].reshape(nblkg, P).max(1)
    # blocks are in degree order; deal round-robin: step j gets blocks
    # j*8 .. j*8+7, K_j = max over them (tight since sorted)
    K = kblk.reshape(nblk, N_CORES).max(1).astype(np.int64)
    K = np.maximum(K, 1)
    KH = -(-K // 2)                              # A/B pair-slots per block
    CW = 2 * KH * P                              # xseq columns per block
    cbase = np.concatenate([[0], np.cumsum(CW)])
    Ltot = int(cbase[-1])

    # edges sorted by destination; starts[d] = first edge of dst d
    e_order = np.argsort(col, kind="stable")
    csrc = row[e_order]
    starts = np.searchsorted(col[e_order], np.arange(npad))

    # per-(step, slot) tables, slot s of block j lives at xseq column
    #   cbase[j] + (sp // 4) * 1024 + half * wt + (sp % 4) * 128 + p
    # where sp = s if s < KH[j] (A half) else s - KH[j] (B half) and wt is
    # the chunk width (512, except the last partial chunk of a block)
    SKtot = int(K.sum())
    row_j = np.repeat(np.arange(nblk), K)            # [SKtot]
    row_s = np.arange(SKtot) - np.repeat(np.cumsum(K) - K, K)
    khj = KH[row_j]
    half = (row_s >= khj).astype(np.int64)
    sp = row_s - half * khj                          # pair-slot index
    wt = np.minimum(MMF, (khj - (sp // 4) * 4) * P)  # chunk width
    colpos = cbase[row_j] + (sp // 4) * 1024 + half * wt + (sp % 4) * P

    # dis-prescaled, transposed x with a zero pad column at index n
    xs_T = np.zeros((in_dim, n + 1), ml_dtypes.bfloat16)
    xs_T[:, :n] = (x.T * dis[None, :]).astype(ml_dtypes.bfloat16)

    dis_pad = np.zeros(npad, np.float32)
    dis_pad[:n] = dis

    has_bi = bool(np.any(np.asarray(b_i)))
    has_bh = bool(np.any(np.asarray(b_h)))

    in_maps = []
    gbs = []
    for c in range(N_CORES):
        gb = np.arange(nblk) * N_CORES + c           # global block ids
        gbs.append(gb)
        dsts = dst_gp[gb]                            # [nblk, P]
        dst_mat = dsts[row_j]                        # [SKtot, P]
        deg_mat = indeg[dst_mat]
        mask = row_s[:, None] < deg_mat              # valid slot?
        eidx = starts[dst_mat] + row_s[:, None]
        seq = np.where(mask, csrc[np.minimum(eidx, len(csrc) - 1)], n)
        seq_cols = np.full(Ltot, n, np.int64)        # default: zero pad col
        seq_cols[(colpos[:, None] + np.arange(P)).reshape(-1)] = seq.reshape(-1)
        xseq = np.ascontiguousarray(xs_T[:, seq_cols])
        dRow = dis_pad[dsts].reshape(1, nblk * P).astype(np.float32)
        m = {"xseq": xseq, "dRow": np.ascontiguousarray(dRow)}
        if has_bi:
            dseq_e = np.where(mask, dis[np.minimum(seq, n - 1)], 0.0)
            dseq = np.zeros(Ltot, np.float32)
            dseq[(colpos[:, None] + np.arange(P)).reshape(-1)] = dseq_e.reshape(-1)
            m["disSeq"] = dseq.reshape(1, Ltot)
        if has_bh:
            with np.errstate(divide="ignore"):
                invd = np.where(dRow > 0, 1.0 / np.maximum(dRow, 1e-30), 0.0)
            m["invdRow"] = invd.astype(np.float32)
        in_maps.append(m)

    W_ch = (np.asarray(W_c, np.float64) @ np.asarray(W_h, np.float64))
    Wi64 = np.asarray(W_i, np.float64)
    W_iA = np.concatenate([Wi64, np.zeros_like(Wi64)], axis=1)  # [W_i | 0]
    W_iB = np.concatenate([np.zeros_like(Wi64), Wi64], axis=1)  # [0 | W_i]
    W_chAB = np.concatenate([W_ch, W_ch], axis=0)               # [W_ch ; W_ch]
    shared = {
        "W_iA": np.ascontiguousarray(W_iA).astype(ml_dtypes.bfloat16),
        "W_iB": np.ascontiguousarray(W_iB).astype(ml_dtypes.bfloat16),
        "W_chAB": np.ascontiguousarray(W_chAB).astype(ml_dtypes.bfloat16),
        "W_o": np.asarray(W_o).astype(ml_dtypes.bfloat16),
    }
    if has_bi:
        shared["b_i"] = np.asarray(b_i, np.float32).reshape(1, hid)
    if has_bh:
        shared["b_h"] = np.asarray(b_h, np.float32).reshape(1, hid)
    for m in in_maps:
        m.update(shared)

    meta = {
        "n": n,
        "npad": npad,
        "nblk": nblk,
        "K": K,
        "KH": KH,
        "cbase": cbase,
        "Ltot": Ltot,
        "in_dim": in_dim,
        "hid": hid,
        "dst_gp": dst_gp,
        "gbs": gbs,
        "has_bi": has_bi,
        "has_bh": has_bh,
        "b_o": float(np.asarray(b_o).reshape(-1)[0]),
    }
    return in_maps, meta


def _build(meta):
    nblk = meta["nblk"]
    KH = meta["KH"]
    cbase = meta["cbase"]
    Ltot = meta["Ltot"]
    in_dim = meta["in_dim"]
    hid = meta["hid"]
    has_bi = meta["has_bi"]
    has_bh = meta["has_bh"]
    b_o = meta["b_o"]
    khmax = int(KH.max())
    NO = nblk * P                                  # output columns

    nc = bacc.Bacc()
    xseq = nc.declare_dram_parameter("xseq", [in_dim, Ltot], BF16, isOutput=False)
    W_iA = nc.declare_dram_parameter("W_iA", [in_dim, 2 * hid], BF16, isOutput=False)
    W_iB = nc.declare_dram_parameter("W_iB", [in_dim, 2 * hid], BF16, isOutput=False)
    W_chAB = nc.declare_dram_parameter("W_chAB", [2 * hid, hid], BF16,
                                       isOutput=False)
    W_o = nc.declare_dram_parameter("W_o", [hid, 1], BF16, isOutput=False)
    dRow = nc.declare_dram_parameter("dRow", [1, NO], F32, isOutput=False)
    if has_bi:
        b_i = nc.declare_dram_parameter("b_i", [1, hid], F32, isOutput=False)
        disSeq = nc.declare_dram_parameter("disSeq", [1, Ltot], F32, isOutput=False)
    if has_bh:
        b_h = nc.declare_dram_parameter("b_h", [1, hid], F32, isOutput=False)
        invdRow = nc.declare_dram_parameter("invdRow", [1, NO], F32, isOutput=False)
    out = nc.declare_dram_parameter("out", [1, NO], F32, isOutput=True)

    with tile.TileContext(nc) as tc, ExitStack() as ctx:
        singles = ctx.enter_context(tc.tile_pool(name="singles", bufs=1))
        sWiA = singles.tile([in_dim, 2 * hid], BF16)
        sWiB = singles.tile([in_dim, 2 * hid], BF16)
        sWch = singles.tile([2 * hid, hid], BF16)
        sWo = singles.tile([hid, 1], BF16)
        sdR = singles.tile([1, NO], F32)
        zall = singles.tile([2 * hid, NO], BF16)
        outrow = singles.tile([1, NO], F32)
        loads = [(sWiA, W_iA), (sWiB, W_iB), (sWch, W_chAB), (sWo, W_o),
                 (sdR, dRow)]
        if has_bi:
            sbi = singles.tile([1, hid], F32)
            sdis = singles.tile([1, Ltot], F32)
            loads += [(sbi, b_i), (sdis, disSeq)]
        if has_bh:
            sbh = singles.tile([1, hid], F32)
            sinvd = singles.tile([1, NO], F32)
            loads += [(sbh, b_h), (sinvd, invdRow)]
        for dst_t, src_t in loads:
            nc.sync.dma_start(out=dst_t[:], in_=src_t[:])

        with (
            tc.tile_pool(name="px", bufs=5) as px,
            tc.tile_pool(name="pps", bufs=4, space="PSUM") as pps,
            tc.tile_pool(name="pag", bufs=3) as pag,
            tc.tile_pool(name="ph", bufs=2) as ph,
            tc.tile_pool(name="ps2", bufs=2, space="PSUM") as ps2,
            tc.tile_pool(name="pso", bufs=2, space="PSUM") as pso,
        ):
            def tail_chunk(t):
                # tail over 4 blocks: W_ch (+A/B merge), relu, W_o, *dis
                w = min(MMF, NO - t)
                p2 = ps2.tile([hid, MMF], F32)
                nc.tensor.matmul(p2[:, :w], lhsT=sWch[:], rhs=zall[:, t: t + w],
                                 start=True, stop=not has_bh)
                if has_bh:
                    nc.tensor.matmul(p2[:, :w], lhsT=sbh[:],
                                     rhs=sinvd[:, t: t + w],
                                     start=False, stop=True)
                h2 = ph.tile([hid, MMF], BF16)
                nc.scalar.activation(h2[:, :w], p2[:, :w], AF.Relu, bias=0.0)
                po = pso.tile([1, MMF], F32)
                nc.tensor.matmul(po[:, :w], lhsT=sWo[:], rhs=h2[:, :w],
                                 start=True, stop=True)
                nc.vector.tensor_mul(outrow[:, t: t + w], po[:, :w],
                                     sdR[:, t: t + w])
                if b_o != 0.0:
                    nc.vector.tensor_scalar_add(
                        outrow[:, t: t + w], outrow[:, t: t + w], b_o,
                    )

            for j in range(nblk):
                KHj = int(KH[j])
                off = int(cbase[j])
                L = 2 * KHj * P                    # block columns (A+B)
                xb = px.tile([in_dim, 2 * khmax * P], BF16, tag="xb")
                dmae = (nc.sync, nc.scalar, nc.gpsimd)[j % 3]
                dmae.dma_start(out=xb[:, :L], in_=xseq[:, off: off + L])
                # Pool cannot read PSUM, so the STT chunks stay on DVE; the
                # SBUF-only collapse ops go to the otherwise-idle Pool engine
                ve = nc.gpsimd
                AG = pag.tile([P, MMF], BF16, tag="ag")
                nchunk = -(-KHj // 4)
                for t in range(nchunk):
                    w = min(MMF, KHj * P - t * MMF)
                    ca = t * 1024                  # A cols of this chunk
                    ps = pps.tile([P, MMF], F32)
                    nc.tensor.matmul(
                        ps[:, :w], lhsT=sWiA[:], rhs=xb[:, ca: ca + w],
                        start=True, stop=False,
                    )
                    nc.tensor.matmul(
                        ps[:, :w], lhsT=sWiB[:],
                        rhs=xb[:, ca + w: ca + 2 * w],
                        start=False, stop=not has_bi,
                    )
                    if has_bi:
                        # rank-1 bias: A then B half (disSeq is column-matched)
                        nc.tensor.matmul(
                            ps[:, :w], lhsT=sbi[:],
                            rhs=sdis[:, off + ca: off + ca + w],
                            start=False, stop=False,
                        )
                        nc.tensor.matmul(
                            ps[:, :w], lhsT=sbi[:],
                            rhs=sdis[:, off + ca + w: off + ca + 2 * w],
                            start=False, stop=True,
                        )
                    if t == 0:
                        nc.scalar.activation(AG[:, :w], ps[:, :w],
                                             AF.Relu, bias=0.0)
                    else:
                        nc.vector.scalar_tensor_tensor(
                            AG[:, :w], ps[:, :w], 0.0, AG[:, :w],
                            op0=ALU.max, op1=ALU.add,
                        )
                # collapse AG's remaining (up to 4) slot-columns; the last
                # halving step writes straight into zall
                zsl = zall[:, j * P: (j + 1) * P]
                k = min(KHj, 4)
                while k > 2:
                    k2 = k // 2
                    h = k - k2
                    ve.tensor_add(
                        AG[:, : k2 * P], AG[:, : k2 * P],
                        AG[:, h * P: (h + k2) * P],
                    )
                    k = h
                if k == 2:
                    ve.tensor_add(zsl, AG[:, :P], AG[:, P: 2 * P])
                else:
                    ve.tensor_copy(zsl, AG[:, :P])
                if j % 4 == 3:
                    tail_chunk((j - 3) * P)
            if nblk % 4 != 0:
                tail_chunk((nblk - nblk % 4) * P)
        nc.sync.dma_start(out=out[:], in_=outrow[:])

    nc.finalize()
    return nc


def _assemble(results, meta):
    n = meta["n"]
    npad = meta["npad"]
    nblk = meta["nblk"]
    dst_gp = meta["dst_gp"]
    out_full = np.zeros(npad, np.float32)
    for c in range(N_CORES):
        vals = np.asarray(results[c]["out"]).reshape(nblk * P)
        out_full[dst_gp[meta["gbs"][c]].ravel()] = vals
    return out_full[:n].reshape(n, 1).astype(np.float32)


def kernel(x, edge_index, W_i, b_i, W_c, W_h, b_h, W_o, b_o):
    x = np.asarray(x)
    edge_index = np.asarray(edge_index)
    in_maps, meta = _host_prep(
        x, edge_index,
        np.asarray(W_i), np.asarray(b_i), np.asarray(W_c),
        np.asarray(W_h), np.asarray(b_h), np.asarray(W_o), np.asarray(b_o),
    )
    nc = _build(meta)
    res = run_bass_kernel_spmd(nc, in_maps, list(range(N_CORES)))
    return _assemble(res.results, meta)
